# revision 4
# baseline (speedup 1.0000x reference)
"""Trainium2 Bass kernel for nn_AE_spikes (spiking autoencoder, 16-step scan).

Data-parallel over 8 NeuronCores: batch 16384 -> 2048 rows/core.

V3 design (vs V2 at ~768us):
  - Encoder runs TWO steps ahead (enc(t+2) emitted in step t) so L1(t+1)
    is emittable from step start: PE always has filler during spike waits.
  - Wide s4 spike computation split: ACT sigmoid on cols [0:S4A),
    DVE tensor_scalar is_ge on cols [S4A:CW) - kills the ACT FIFO
    head-of-line blocking that stalled s1/s2/s3 (and thus the PE) in V2.
  - s1(t+1) emitted at the tail of step t (before the s4 pieces) so
    L2(t+1) never waits on ACT backlog.
  - cp1 += s0 moved to GpSimd (one full step of slack; DVE port
    contention acceptable there).
  - Finish: final-reset stationary rst4f carries bias row -16*b4f16 so
    out = -pv4/16 exactly -> one ACT piece + one DVE piece, no per-chunk
    bias ACTs; b4v table and dead W4-lo weights dropped.
  - Membranes PSUM-resident as in V2: pv123 [128,1024] (2 banks, bufs=2),
    pv4 [128,2048] (4 banks, bufs=1). Weight matmuls fp16 hi/lo (exact
    products with binary spikes; precision study: no lo can be dropped).
"""

import os
import sys

import numpy as np

if "/opt/trn_rl_repo" not in sys.path:
    sys.path.insert(0, "/opt/trn_rl_repo")

B = 16384
IN = 784
H = 128
T = 16
NCORES = 8
BC = B // NCORES          # 2048 batch rows per core
NT = 256                  # batch-tile columns
NTILES = BC // NT         # 8
CH = 7                    # feature chunks of 112 rows
KC = 112
CW = CH * NT              # concatenated width, 1792
KAPPA = float(2 ** 30)    # sigmoid-step scale
S4A = 1024                # s4 cols computed on ACT (rest on DVE)
EH1 = 1024                # encoder first-half cols (chunks 0..3)

LAST_RESULT = None
_CACHE = {}


def _install_ntff_shim():
    """Make run_bass_kernel_spmd(trace=True) work in this container."""
    import types

    try:
        from antenv.axon_hooks import get_axon_ntff_profile_hook  # noqa: F401
        return
    except ImportError:
        pass
    try:
        import antenv
        from trn_agent_boot.trn_boot import _ntff_profile_via_ctypes
    except ImportError:
        return
    mod = types.ModuleType("antenv.axon_hooks")
    mod._hook = _ntff_profile_via_ctypes("/opt/axon/libaxon_pjrt.so")
    mod.set_axon_ntff_profile_hook = lambda h: setattr(mod, "_hook", h)
    mod.get_axon_ntff_profile_hook = lambda: mod._hook
    sys.modules["antenv.axon_hooks"] = mod
    antenv.axon_hooks = mod


def _build():
    import concourse.tile as tile
    from concourse import bacc, mybir
    from contextlib import ExitStack

    f32 = mybir.dt.float32
    f16 = mybir.dt.float16
    Alu = mybir.AluOpType
    ActF = mybir.ActivationFunctionType

    nc = bacc.Bacc("TRN2", target_bir_lowering=False, debug=False)

    fT_d = nc.dram_tensor("fT", [IN, BC], f32, kind="ExternalInput").ap()
    # Weights as fp16 hi/lo splits (products with binary spikes are
    # fp32-exact in PSUM; hi-only drifts chaotically - measured 0.09 rel).
    # w1s: [113, 128] per chunk, concatenated along free dim: chunk c at
    # cols [c*H, (c+1)*H). Row 112 of chunk 0 is b1 (hi/lo split across
    # the two stationaries); rows 112 of chunks 1..6 are zero.
    w1sh_d = nc.dram_tensor("w1sh", [KC + 1, CH * H], f16, kind="ExternalInput").ap()
    w1sl_d = nc.dram_tensor("w1sl", [KC + 1, CH * H], f16, kind="ExternalInput").ap()
    w2h_d = nc.dram_tensor("w2h", [H, H], f16, kind="ExternalInput").ap()
    w2l_d = nc.dram_tensor("w2l", [H, H], f16, kind="ExternalInput").ap()
    w3h_d = nc.dram_tensor("w3h", [H, H], f16, kind="ExternalInput").ap()
    w3l_d = nc.dram_tensor("w3l", [H, H], f16, kind="ExternalInput").ap()
    w4Th_d = nc.dram_tensor("w4Th", [H, IN], f16, kind="ExternalInput").ap()
    nw4Th_d = nc.dram_tensor("nw4Th", [H, IN], f16, kind="ExternalInput").ap()
    # rst4: [113, 112] per chunk (cols c*KC..): rows 0..111 = -I, row 112 = b4f16
    rst4_d = nc.dram_tensor("rst4", [KC + 1, IN], f16, kind="ExternalInput").ap()
    # rst4f: final reset, row 112 = -16*b4f16 (cancels the 16 bias adds so
    # out = -pv4/16 with no bias term)
    rst4f_d = nc.dram_tensor("rst4f", [KC + 1, IN], f16, kind="ExternalInput").ap()
    negI_d = nc.dram_tensor("negI", [H, H], f16, kind="ExternalInput").ap()
    thh_d = nc.dram_tensor("thh", [H, 2 * T], f32, kind="ExternalInput").ap()
    out_d = nc.dram_tensor("outT", [IN, BC], f32, kind="ExternalOutput").ap()

    with tile.TileContext(nc) as tc:
        with ExitStack() as ctx:
            wp = ctx.enter_context(tc.tile_pool(name="weights", bufs=1))
            fp = ctx.enter_context(tc.tile_pool(name="feat", bufs=2))
            cpp = ctx.enter_context(tc.tile_pool(name="cp1p", bufs=2))
            shp = ctx.enter_context(tc.tile_pool(name="shid", bufs=6))
            s3ap = ctx.enter_context(tc.tile_pool(name="s3ap", bufs=2))
            outp = ctx.enter_context(tc.tile_pool(name="outp", bufs=2))
            s0p = ctx.enter_context(tc.tile_pool(name="s0p", bufs=1))
            s4p = ctx.enter_context(tc.tile_pool(name="s4p", bufs=1))
            pv123p = ctx.enter_context(
                tc.tile_pool(name="pv123", bufs=2, space="PSUM"))
            pv4p = ctx.enter_context(
                tc.tile_pool(name="pv4", bufs=1, space="PSUM"))

            # ---- load weights / tables once ----
            def wload(name, dram, shape):
                tl = wp.tile(shape, f16, tag=name, name=name)
                nc.sync.dma_start(tl[:], dram[:])
                return tl

            w1sh = wload("w1sh", w1sh_d, [KC + 1, CH * H])
            w1sl = wload("w1sl", w1sl_d, [KC + 1, CH * H])
            w2h = wload("w2h", w2h_d, [H, H])
            w2l = wload("w2l", w2l_d, [H, H])
            w3h = wload("w3h", w3h_d, [H, H])
            w3l = wload("w3l", w3l_d, [H, H])
            w4Th = wload("w4Th", w4Th_d, [H, IN])
            nw4Th = wload("nw4Th", nw4Th_d, [H, IN])
            rst4 = wload("rst4", rst4_d, [KC + 1, IN])
            rst4f = wload("rst4f", rst4f_d, [KC + 1, IN])
            negI = wload("negI", negI_d, [H, H])
            thh = wp.tile([H, 2 * T], f32, tag="thh")
            nc.sync.dma_start(thh[:], thh_d[:])
            nk1 = wp.tile([H, 1], f32, tag="nk1")
            nc.gpsimd.memset(nk1[:], -KAPPA)

            # persistent spike buffers (rotating by step mod 3), with the
            # constant-1 row at partition 112 (bias row for K=113 matmuls)
            s0b = []
            s4b = []
            for i in range(3):
                # ones row lives at partition 112; memset must start at a
                # quadrant boundary, so fill [96:113] then let the per-step
                # writes to [0:112] overwrite the data rows.
                t0 = s0p.tile([KC + 1, CW], f16, tag=f"s0_{i}", name=f"s0_{i}")
                nc.gpsimd.memset(t0[96:KC + 1, :], 1.0)
                s0b.append(t0)
                t4 = s4p.tile([KC + 1, CW], f16, tag=f"s4_{i}", name=f"s4_{i}")
                nc.gpsimd.memset(t4[96:KC + 1, :], 1.0)
                s4b.append(t4)

            def enc_half(fTt, cp1, t, dst, c0, c1):
                """s0_t cols [c0:c1) = ((t+1)*f >= cp1) on DVE."""
                nc.vector.scalar_tensor_tensor(
                    dst[0:KC, c0:c1], fTt[:, c0:c1], float(t + 1),
                    cp1[:, c0:c1], Alu.mult, Alu.is_ge)

            def fT_load(b):
                fTt = fp.tile([KC, CW], f32, tag="fT", name=f"fT_{b}")
                c0 = b * NT
                for c in range(CH):
                    nc.sync.dma_start(
                        fTt[:, c * NT:(c + 1) * NT],
                        fT_d[KC * c:KC * (c + 1), c0:c0 + NT])
                return fTt

            fTt_next = fT_load(0)
            for b in range(NTILES):
                c0 = b * NT
                fTt = fTt_next
                if b + 1 < NTILES:
                    fTt_next = fT_load(b + 1)  # prefetch next tile early
                pv123 = pv123p.tile([H, 1024], f32, tag="pv123", name=f"pv123_{b}")
                v1 = pv123[:, 0:NT]
                v2 = pv123[:, NT:2 * NT]
                v3 = pv123[:, 2 * NT:3 * NT]
                pv4 = pv4p.tile([H, 2048], f32, tag="pv4", name=f"pv4_{b}")
                s3a = s3ap.tile([H, NT], f16, tag="s3a", name=f"s3a_{b}")
                nc.vector.memset(s3a[:], 0.0)
                # s4[-1] := 0 (its -I contribution at t=0 must vanish; the
                # b4 bias row still fires, initializing v4 to b4)
                nc.gpsimd.memset(s4b[2][0:KC, :], 0.0)
                cp1 = cpp.tile([KC, CW], f16, tag="cp1", name=f"cp1_{b}")
                nc.gpsimd.memset(cp1[:], 1.0)

                def emit_L1(t, s0, chunks):
                    """Weight matmuls of layer 1 (hi/lo per chunk + bias
                    row on chunk 0)."""
                    for c in chunks:
                        kc = KC + 1 if c == 0 else KC
                        rhs = s0[0:kc, c * NT:(c + 1) * NT]
                        nc.tensor.matmul(
                            v1, w1sh[0:kc, c * H:(c + 1) * H], rhs,
                            start=(t == 0 and c == 0), stop=False,
                            skip_group_check=True)
                        nc.tensor.matmul(
                            v1, w1sl[0:kc, c * H:(c + 1) * H], rhs,
                            start=False,
                            stop=(t == T - 1 and c == CH - 1),
                            skip_group_check=True)

                def emit_rst4(t, s4_prev, chunks):
                    for c in chunks:
                        dst = pv4[0:KC, c * NT:(c + 1) * NT]
                        # one start=True per psum bank (chunks pair 2-per-
                        # bank; odd chunks open implicitly - start=True
                        # clears has_written for the WHOLE bank)
                        nc.tensor.matmul(
                            dst, rst4[:, c * KC:(c + 1) * KC],
                            s4_prev[:, c * NT:(c + 1) * NT],
                            start=(t == 0 and c % 2 == 0), stop=False,
                            skip_group_check=True)

                def cp1_add(t, c0, c1):
                    """cp1 += s0(t) cols [c0:c1) on GpSimd (slack path)."""
                    nc.gpsimd.tensor_tensor(
                        cp1[:, c0:c1], cp1[:, c0:c1],
                        s0b[t % 3][0:KC, c0:c1], Alu.add)

                # ---- prologue: enc(0), cp1+=s0(0), enc(1), L1(0), s1(0) ----
                enc_half(fTt, cp1, 0, s0b[0], 0, EH1)
                enc_half(fTt, cp1, 0, s0b[0], EH1, CW)
                cp1_add(0, 0, EH1)
                enc_half(fTt, cp1, 1, s0b[1], 0, EH1)
                cp1_add(0, EH1, CW)
                enc_half(fTt, cp1, 1, s0b[1], EH1, CW)
                emit_L1(0, s0b[0], range(CH))
                s1_cur = shp.tile([H, NT], f16, tag="s1")
                nc.scalar.activation(s1_cur[:], v1, ActF.Sigmoid,
                                     bias=nk1[:], scale=KAPPA)

                for t in range(T):
                    s4_prev = s4b[(t - 1) % 3]
                    s4 = s4b[t % 3]
                    # -------- layer 2 (v2 shares v1's bank: no start) ----
                    nc.tensor.matmul(v2, w2h[:], s1_cur[:], start=False,
                                     stop=False, skip_group_check=True)
                    nc.tensor.matmul(v2, w2l[:], s1_cur[:], start=False,
                                     stop=(t == T - 1), skip_group_check=True)
                    # reset v1 right after its spike was read
                    if t < T - 1:
                        nc.tensor.matmul(v1, negI[:], s1_cur[:], start=False,
                                         stop=False, skip_group_check=True)
                    # L1(t+1): emittable immediately (enc ran 2 steps ahead)
                    if t + 1 < T:
                        emit_L1(t + 1, s0b[(t + 1) % 3], range(CH))
                    # rst4(t): s4(t-1) finished ~1us into this step; sits
                    # behind L1(t+1) in the PE queue by design
                    emit_rst4(t, s4_prev, range(CH))
                    # encoder for step t+2 + cp1 catch-up for t+1, halves
                    # pipelined so enc_h1 only waits on the h1 add
                    if t + 2 < T:
                        cp1_add(t + 1, 0, EH1)
                        enc_half(fTt, cp1, t + 2, s0b[(t + 2) % 3], 0, EH1)
                        cp1_add(t + 1, EH1, CW)
                        enc_half(fTt, cp1, t + 2, s0b[(t + 2) % 3], EH1, CW)
                    s2 = shp.tile([H, NT], f16, tag="s2")
                    nc.scalar.activation(s2[:], v2, ActF.Sigmoid,
                                         bias=thh[:, t:t + 1], scale=KAPPA)

                    # -------- layer 3 --------
                    nc.tensor.matmul(v3, w3h[:], s2[:], start=(t == 0),
                                     stop=False, skip_group_check=True)
                    nc.tensor.matmul(v3, w3l[:], s2[:], start=False,
                                     stop=(t == T - 1), skip_group_check=True)
                    if t < T - 1:
                        nc.tensor.matmul(v2, negI[:], s2[:], start=False,
                                         stop=False, skip_group_check=True)
                    s3 = shp.tile([H, NT], f16, tag="s3")
                    nc.scalar.activation(s3[:], v3, ActF.Sigmoid,
                                         bias=thh[:, T + t:T + t + 1], scale=KAPPA)

                    # -------- layer 4 weight matmuls --------
                    for c in range(CH):
                        dst = pv4[0:KC, c * NT:(c + 1) * NT]
                        nc.tensor.matmul(
                            dst, w4Th[:, c * KC:(c + 1) * KC], s3[:],
                            start=False, stop=False, skip_group_check=True)
                    if t < T - 1:
                        nc.tensor.matmul(v3, negI[:], s3[:], start=False,
                                         stop=False, skip_group_check=True)

                    # s1 for the NEXT step, emitted before the s4 pieces so
                    # L2(t+1) never queues behind them on ACT
                    if t + 1 < T:
                        s1_cur = shp.tile([H, NT], f16, tag="s1")
                        nc.scalar.activation(s1_cur[:], v1, ActF.Sigmoid,
                                             bias=nk1[:], scale=KAPPA)
                    # s4(t): ACT piece + DVE piece
                    nc.scalar.activation(s4[0:KC, 0:S4A], pv4[0:KC, 0:S4A],
                                         ActF.Sigmoid, bias=nk1[0:KC, :],
                                         scale=KAPPA)
                    nc.vector.tensor_scalar(s4[0:KC, S4A:CW],
                                            pv4[0:KC, S4A:CW], 1.0, None,
                                            Alu.is_ge)
                    # S3 accumulation (fp16 exact, max 16)
                    nc.vector.tensor_tensor(s3a[:], s3a[:], s3[:], Alu.add)

                # ---- finish tile: count4 = -(v4_final')  [biases cancel] --
                s4_last = s4b[(T - 1) % 3]
                out = outp.tile([KC, CW], f32, tag="out")
                for c in range(CH):
                    dst = pv4[0:KC, c * NT:(c + 1) * NT]
                    nc.tensor.matmul(
                        dst, rst4f[:, c * KC:(c + 1) * KC],
                        s4_last[:, c * NT:(c + 1) * NT],
                        start=False, stop=False, skip_group_check=True)
                    nc.tensor.matmul(
                        dst, nw4Th[:, c * KC:(c + 1) * KC], s3a[:],
                        start=False, stop=True, skip_group_check=True)
                # out = -pv4/16 = count/16 : ACT piece + DVE piece
                nc.scalar.activation(out[:, 0:S4A], pv4[0:KC, 0:S4A],
                                     ActF.Copy, bias=0.0, scale=-1.0 / 16.0)
                nc.vector.tensor_scalar(out[:, S4A:CW], pv4[0:KC, S4A:CW],
                                        -1.0 / 16.0, None, Alu.mult)
                for c in range(CH):
                    nc.sync.dma_start(
                        out_d[KC * c:KC * (c + 1), c0:c0 + NT],
                        out[:, c * NT:(c + 1) * NT])

    nc.compile()
    return nc


def _host_prep(inputs):
    f32 = np.float32
    f16 = np.float16
    features = np.asarray(inputs["features"], f32)
    fT = np.ascontiguousarray(features.T)  # [784, 16384]

    b1 = np.asarray(inputs["b1"], f32)
    b2 = np.asarray(inputs["b2"], f32)
    b3 = np.asarray(inputs["b3"], f32)
    b4 = np.asarray(inputs["b4"], f32)

    def split(w):
        hi = w.astype(f16)
        lo = (w - hi.astype(f32)).astype(f16)
        return hi, lo

    w1T = np.asarray(inputs["W1"], f32).T               # [784, 128]
    b1h, b1l = split(b1)
    w1Th, w1Tl = split(w1T)
    w1sh = np.zeros((KC + 1, CH * H), f16)
    w1sl = np.zeros((KC + 1, CH * H), f16)
    for c in range(CH):
        w1sh[0:KC, c * H:(c + 1) * H] = w1Th[c * KC:(c + 1) * KC, :]
        w1sl[0:KC, c * H:(c + 1) * H] = w1Tl[c * KC:(c + 1) * KC, :]
    w1sh[KC, 0:H] = b1h
    w1sl[KC, 0:H] = b1l

    w2h, w2l = split(np.ascontiguousarray(np.asarray(inputs["W2"], f32).T))
    w3h, w3l = split(np.ascontiguousarray(np.asarray(inputs["W3"], f32).T))
    w4Th, _w4Tl = split(np.ascontiguousarray(np.asarray(inputs["W4"], f32).T))
    nw4Th = (-w4Th).astype(f16)

    b4f = b4.astype(f16)
    rst4 = np.zeros((KC + 1, IN), f16)
    rst4f = np.zeros((KC + 1, IN), f16)
    for c in range(CH):
        sl = slice(c * KC, (c + 1) * KC)
        rst4[0:KC, sl] = -np.eye(KC, dtype=f16)
        rst4[KC, sl] = b4f[sl]
        rst4f[0:KC, sl] = -np.eye(KC, dtype=f16)
        # cancels the 16 b4f16 adds (t=0 init + 15 per-step resets):
        # -16*b4f16 is exact in fp16 (scale by 2^4)
        rst4f[KC, sl] = (-16.0 * b4f[sl].astype(f32)).astype(f16)

    negI = (-np.eye(H)).astype(f16)

    # thresholds for layers 2,3: th[o,t] = 1 - cumsum(b)[t+1 terms], as
    # -KAPPA*th for the sigmoid bias. Iterative fp32 cumsum. v2 at step t
    # (0-based) has had (t+1) bias adds folded out.
    def cum_th(bv):
        c = np.zeros_like(bv)
        th = np.empty((bv.shape[0], T), f32)
        for t in range(T):
            c = (c + bv.astype(f32)).astype(f32)
            th[:, t] = (np.float32(1.0) - c).astype(f32)
        return th

    thh = np.concatenate([cum_th(b2), cum_th(b3)], axis=1)  # [128, 32]
    nkthh = (-KAPPA * thh).astype(f32)

    shared = {
        "w1sh": np.ascontiguousarray(w1sh),
        "w1sl": np.ascontiguousarray(w1sl),
        "w2h": w2h, "w2l": w2l, "w3h": w3h, "w3l": w3l,
        "w4Th": w4Th, "nw4Th": nw4Th,
        "rst4": np.ascontiguousarray(rst4),
        "rst4f": np.ascontiguousarray(rst4f),
        "negI": negI,
        "thh": np.ascontiguousarray(nkthh),
    }
    in_maps = []
    for i in range(NCORES):
        m = dict(shared)
        m["fT"] = np.ascontiguousarray(fT[:, i * BC:(i + 1) * BC])
        in_maps.append(m)
    return in_maps


def kernel(**inputs):
    global LAST_RESULT
    if os.environ.get("BASS_TRACE"):
        _install_ntff_shim()
    from concourse.bass_utils import run_bass_kernel_spmd

    if "nc" not in _CACHE:
        _CACHE["nc"] = _build()
    nc = _CACHE["nc"]

    in_maps = _host_prep(inputs)
    kwargs = {}
    if os.environ.get("BASS_TRACE"):
        kwargs["tmpdir"] = os.environ.get("BASS_TRACE_DIR") or None
    try:
        res = run_bass_kernel_spmd(nc, in_maps, core_ids=list(range(NCORES)), **kwargs)
    except Exception:
        # transient device faults usually clear on retry
        import time

        time.sleep(2)
        res = run_bass_kernel_spmd(nc, in_maps, core_ids=list(range(NCORES)), **kwargs)
    LAST_RESULT = res

    outT = np.concatenate([res.results[i]["outT"] for i in range(NCORES)], axis=1)
    return np.ascontiguousarray(outT.T).astype(np.float32)
